# revision 23
# baseline (speedup 1.0000x reference)
"""DiskLoss Trainium2 kernel (interval-union formulation).

Computes the reference loss:
  pred = gather(output, ind)            # [K,33] per batch
  gt_m = even-odd rasterization of the 16-gon from target   (per object)
  dk_m = union of 15 disks (radius ceil(|pred[:,32]|)) from pred
  per_obj = 1 - inter/(union+1e-6);  loss = sum(m*per_obj)/(sum(m)+1e-6)

Sharding: data-parallel over batch B=8 -> one batch element per NeuronCore.
Each core reduces its 128 objects (object-per-partition layout) to
(sum m*per_obj, sum m); host adds the 8 partial pairs.

Device algorithm — both masks are per-row interval unions, no pixel raster:
  - disks: per (k,y,d) the row span is [s,e) with h=sqrt(relu(r^2-(y-cy)^2)),
    s=max(0,floor(cx-h)+1), e=min(128,floor(cx+h)+1); floor/clip via ACT
    round-to-nearest int conversion (+-0.5 bias) with uint8 saturation;
    pack p=s*129+(e-128) int16, Batcher-sort the 16 slots per row, unpack
    via ACT, prefix-max ends -> disjoint runs [s'_j, u'_j) (values shifted
    by -128); disk area = sum relu(u'-s')
  - polygon: crossings xint per (y in 32:96, v) in fp32; straddle via
    (y-y1)(y-y2)<0; c-128 = round(xint-95.5) via ACT, zeroed (sentinel)
    unless straddle, packed *130 and sorted in the same Batcher pass
    (rows 128:192); even-odd pairs (a_i,b_i) of the sorted crossings are
    the disjoint poly intervals; poly area = sum(b-a)
  - intersection = sum_{i,j} relu(min(b_i,u'_j) - max(a_i,s'_j)) over the
    poly-overlap rows 32:96 (8 poly pairs x 15 disk runs, int16)
  - DVE does min/max/sort; Pool (gpsimd) does arithmetic (add/sub/mult
    only on this ISA); ACT does all float->int rounding; PE does the final
    masked reduction via ones-matmul over partitions
"""

import sys

if "/opt/trn_rl_repo" not in sys.path:
    sys.path.insert(0, "/opt/trn_rl_repo")

import numpy as np

B, C, H, W = 8, 33, 128, 128
K = 128
V = 16          # polygon vertices
D = 15          # disk centers

_CACHE = {}


def _build_nc():
    import concourse.bacc as bacc
    import concourse.mybir as mybir
    import concourse.tile as tile
    import concourse.bass as bass

    F32 = mybir.dt.float32
    F16 = mybir.dt.float16
    I32 = mybir.dt.int32
    I16 = mybir.dt.int16
    U8 = mybir.dt.uint8
    Alu = mybir.AluOpType
    Act = mybir.ActivationFunctionType

    nc = bacc.Bacc("TRN2", target_bir_lowering=False, debug=False)

    # ---- DRAM I/O (per core) ----
    featT_d = nc.dram_tensor("featT", [H * W, C], F32, kind="ExternalInput")
    ind_d = nc.dram_tensor("ind", [K], I32, kind="ExternalInput")
    tgt_d = nc.dram_tensor("target", [K, C], F32, kind="ExternalInput")
    mask_d = nc.dram_tensor("mask", [K], I32, kind="ExternalInput")
    out_d = nc.dram_tensor("out", [2], F32, kind="ExternalOutput")

    # ---- SBUF ----
    pred = nc.alloc_sbuf_tensor("pred", [K, C], F32)
    tgt = nc.alloc_sbuf_tensor("tgt", [K, C], F32)
    indc = nc.alloc_sbuf_tensor("indc", [K, 1], I32)
    maski = nc.alloc_sbuf_tensor("maski", [K, 1], I32)
    maskf = nc.alloc_sbuf_tensor("maskf", [K, 1], F32)

    pxi = nc.alloc_sbuf_tensor("pxi", [K, W], I32)
    pyg = nc.alloc_sbuf_tensor("pyg", [K, W], F32)      # y global 0..127
    pysh = nc.alloc_sbuf_tensor("pysh", [K, 64], F32)   # y-32 for rows 32:96

    # disk geometry
    negcu = nc.alloc_sbuf_tensor("negcu", [K, D], F32)  # -(cy+32)
    cxg = nc.alloc_sbuf_tensor("cxg", [K, D], F32)      # cx+32
    rsc = nc.alloc_sbuf_tensor("rsc", [K, 4], F32)
    ri = nc.alloc_sbuf_tensor("ri", [K, 1], I32)
    r2u = nc.alloc_sbuf_tensor("r2u", [K, 1], F32)
    sqyu = nc.alloc_sbuf_tensor("sqyu", [K, H, D], F32)
    hsq = nc.alloc_sbuf_tensor("hsq", [K, H, D], F32)
    hh = nc.alloc_sbuf_tensor("hh", [K, H, D], F32)
    lo = nc.alloc_sbuf_tensor("lo", [K, H, D], F32)
    hi = nc.alloc_sbuf_tensor("hi", [K, H, D], F32)
    s8 = nc.alloc_sbuf_tensor("s8", [K, H, D], U8)
    e8 = nc.alloc_sbuf_tensor("e8", [K, H, D], U8)

    # polygon geometry
    x2b = nc.alloc_sbuf_tensor("x2b", [K, V], F32)
    y2b = nc.alloc_sbuf_tensor("y2b", [K, V], F32)
    pv1 = nc.alloc_sbuf_tensor("pv1", [K, V], F32)
    pv2 = nc.alloc_sbuf_tensor("pv2", [K, V], F32)
    pv3 = nc.alloc_sbuf_tensor("pv3", [K, V], F32)
    xa = nc.alloc_sbuf_tensor("xa", [K, 64, V], F32)
    ya = nc.alloc_sbuf_tensor("ya", [K, 64, V], F32)
    xb = nc.alloc_sbuf_tensor("xb", [K, 64, V], F32)
    nei = nc.alloc_sbuf_tensor("nei", [K, 64, V], I16)
    cpre = nc.alloc_sbuf_tensor("cpre", [K, 64, V], I16)

    # sort + runs (disk rows int16 on DVE; poly rows f32 on Pool)
    pks = nc.alloc_sbuf_tensor("pks", [K, 128, 16], I16)
    mtA = nc.alloc_sbuf_tensor("mtA", [K, 128, 8], I16)
    pksf = nc.alloc_sbuf_tensor("pksf", [K, 64, 16], F32)
    sumP = nc.alloc_sbuf_tensor("sumP", [K, 64, 8], F32)
    difP = nc.alloc_sbuf_tensor("difP", [K, 64, 8], F32)
    abh = nc.alloc_sbuf_tensor("abh", [K, 64, 8], F32)
    hsm = nc.alloc_sbuf_tensor("hsm", [K, 64, 8], F32)
    sshift = nc.alloc_sbuf_tensor("sshift", [K, H, 16], I16)   # s-128 (disk)
    suc = nc.alloc_sbuf_tensor("suc", [K, 192, 16], I16)       # s | c-128
    etld = nc.alloc_sbuf_tensor("etld", [K, H, 16], I16)       # e-128 sorted
    ebuf = nc.alloc_sbuf_tensor("ebuf", [K, H, 16], I16)
    uu = nc.alloc_sbuf_tensor("uu", [K, H, D], I16)
    dd = nc.alloc_sbuf_tensor("dd", [K, H, D], I16)

    # intersection
    mx = nc.alloc_sbuf_tensor("mx", [K, 64, 8, D], I16)
    mn = nc.alloc_sbuf_tensor("mn", [K, 64, 8, D], I16)
    df = nc.alloc_sbuf_tensor("df", [K, 64, 8, D], I16)
    bE = nc.alloc_sbuf_tensor("bE", [K, 64, 8, D], I16)

    # act bias constants
    bm95 = nc.alloc_sbuf_tensor("bm95", [K, 1], F32)     # -95.5 (poly c shift)
    bp05 = nc.alloc_sbuf_tensor("bp05", [K, 1], F32)     # +0.5
    b1275 = nc.alloc_sbuf_tensor("b1275", [K, 1], F32)   # +127.5

    # reduction
    stats = nc.alloc_sbuf_tensor("stats", [K, 8], F32)
    onesv = nc.alloc_sbuf_tensor("onesv", [K, 1], F32)
    colq = nc.alloc_sbuf_tensor("colq", [K, 2], F32)
    outsb = nc.alloc_sbuf_tensor("outsb", [1, 2], F32)
    psum = nc.alloc_psum_tensor("psum", [1, 2], F32)

    with tile.TileContext(nc) as tc:
        vec = nc.vector
        gps = nc.gpsimd
        act = nc.scalar

        def ts(eng, out, in0, s1, op0, s2=None, op1=None, accum=None):
            kw = {}
            if accum is not None:
                kw["accum_out"] = accum
            if op1 is not None:
                return eng.tensor_scalar(out=out, in0=in0, scalar1=s1, scalar2=s2,
                                         op0=op0, op1=op1, **kw)
            return eng.tensor_scalar(out=out, in0=in0, scalar1=s1, scalar2=None,
                                     op0=op0, **kw)

        def tt(eng, out, in0, in1, op):
            return eng.tensor_tensor(out=out, in0=in0, in1=in1, op=op)

        # ---- P0: input DMAs + gather + iotas + consts ----
        nc.sync.dma_start(indc.ap(), ind_d.ap().unsqueeze(1))
        nc.sync.dma_start(tgt.ap(), tgt_d.ap())
        nc.sync.dma_start(maski.ap(), mask_d.ap().unsqueeze(1))
        nc.gpsimd.indirect_dma_start(
            out=pred.ap(), out_offset=None, in_=featT_d.ap(),
            in_offset=bass.IndirectOffsetOnAxis(ap=indc.ap(), axis=0))

        nc.gpsimd.iota(pxi.ap(), pattern=[[1, W]], base=0, channel_multiplier=0)
        ts(vec, pyg.ap(), pxi.ap(), 0.0, Alu.add)            # int->f32, 0..127
        ts(vec, pysh.ap(), pxi.ap()[:, 32:96], -32.0, Alu.add)
        ts(vec, maskf.ap(), maski.ap(), 0.0, Alu.add)
        vec.memset(bm95.ap(), -95.5)
        vec.memset(bp05.ap(), 0.5)
        vec.memset(b1275.ap(), 127.5)
        vec.memset(onesv.ap(), 1.0)

        # ---- P2a: disk scalars first (unblocks the ACT Square chain) ----
        ts(vec, negcu.ap(), pred.ap()[:, 1:2 * D:2], -1.0, Alu.mult, -32.0, Alu.add)
        ts(vec, cxg.ap(), pred.ap()[:, 0:2 * D:2], 32.0, Alu.add)
        u = rsc.ap()[:, 0:1]; t = rsc.ap()[:, 1:2]; g = rsc.ap()[:, 2:3]
        ts(vec, t, pred.ap()[:, 32:33], -1.0, Alu.mult)
        tt(vec, u, pred.ap()[:, 32:33], t, Alu.max)          # |p|
        vec.tensor_copy(out=ri.ap(), in_=u)
        vec.tensor_copy(out=t, in_=ri.ap())
        tt(vec, g, t, u, Alu.is_gt)
        tt(vec, t, t, g, Alu.subtract)                       # floor
        tt(vec, g, u, t, Alu.is_gt)
        tt(vec, t, t, g, Alu.add)                            # ceil = r
        tt(vec, r2u.ap(), t, t, Alu.mult)                    # r^2

        for d in range(D):
            act.activation(out=sqyu.ap()[:, :, d], in_=pyg.ap(), func=Act.Square,
                           bias=negcu.ap()[:, d:d + 1], scale=1.0)

        # ---- P1: polygon precompute (raw coords; rows are y-32 in 0..63) ----
        x1v = tgt.ap()[:, 0:2 * V:2]
        y1v = tgt.ap()[:, 1:2 * V:2]
        gps.tensor_copy(out=x2b.ap()[:, 0:V - 1], in_=tgt.ap()[:, 2:2 * V:2])
        gps.tensor_copy(out=x2b.ap()[:, V - 1:V], in_=tgt.ap()[:, 0:1])
        gps.tensor_copy(out=y2b.ap()[:, 0:V - 1], in_=tgt.ap()[:, 3:2 * V:2])
        gps.tensor_copy(out=y2b.ap()[:, V - 1:V], in_=tgt.ap()[:, 1:2])
        d0 = pv1.ap(); eqz = pv2.ap(); sl = pv3.ap()
        tt(vec, d0, y2b.ap(), y1v, Alu.subtract)
        ts(vec, eqz, d0, 0.0, Alu.is_equal)
        tt(vec, d0, d0, eqz, Alu.add)                        # denom
        vec.reciprocal(out=eqz, in_=d0)                      # 1/denom
        tt(vec, sl, x2b.ap(), x1v, Alu.subtract)
        tt(vec, sl, sl, eqz, Alu.mult)                       # slope

        pyb = pysh.ap().unsqueeze(2).to_broadcast([K, 64, V])
        y1b = y1v.unsqueeze(1).to_broadcast([K, 64, V])
        y2bb = y2b.ap().unsqueeze(1).to_broadcast([K, 64, V])
        # straddle = (y-y1)(y-y2) < 0  (a.s. equal to reference predicate)
        tt(vec, xa.ap(), pyb, y1b, Alu.subtract)             # y-y1 (also xint)
        tt(gps, ya.ap(), pyb, y2bb, Alu.subtract)            # y-y2
        tt(gps, ya.ap(), ya.ap(), xa.ap(), Alu.mult)
        ts(vec, nei.ap(), ya.ap(), 0.0, Alu.is_lt)           # straddle 0/1 i16
        # xint (raw coords) = x1 + (y - y1)*slope
        tt(vec, xb.ap(), xa.ap(), sl.unsqueeze(1).to_broadcast([K, 64, V]), Alu.mult)
        tt(vec, xa.ap(), xb.ap(), x1v.unsqueeze(1).to_broadcast([K, 64, V]), Alu.add)
        # c-128 = round(xint_raw - 95.5); garbage for non-straddle (zeroed next)
        act.activation(out=cpre.ap(), in_=xa.ap(), func=Act.Identity,
                       bias=bm95.ap(), scale=1.0)
        # pack poly rows (f32, Pool sorts these): p = (c-128)*130 * straddle
        vec.scalar_tensor_tensor(out=pksf.ap(), in0=cpre.ap(),
                                 scalar=130.0, in1=nei.ap(),
                                 op0=Alu.mult, op1=Alu.mult)

        # ---- P2b: disk per-row geometry ----
        # hsqn = min(sqyu - r^2, 0);  h = sqrt(-hsqn)
        ts(vec, hsq.ap(), sqyu.ap(), r2u.ap(), Alu.subtract, 0.0, Alu.min)
        act.activation(out=hh.ap(), in_=hsq.ap(), func=Act.Sqrt,
                       bias=0.0, scale=-1.0)
        cxb = cxg.ap().unsqueeze(1).to_broadcast([K, H, D])
        tt(vec, lo.ap(), cxb, hh.ap(), Alu.subtract)
        tt(vec, hi.ap(), cxb, hh.ap(), Alu.add)
        # s = round(lo+0.5) sat to [0,255];  e_rev = round(127.5-hi) sat
        act.activation(out=s8.ap(), in_=lo.ap(), func=Act.Identity,
                       bias=bp05.ap(), scale=1.0)
        act.activation(out=e8.ap(), in_=hi.ap(), func=Act.Identity,
                       bias=b1275.ap(), scale=-1.0)
        gps.memset(pks.ap()[:, 0:128, D:16], 16512)          # s=128,e=128 sentinel

        # ---- P3: Batcher odd-even mergesort of the 16 slots (all rows) ----
        GROUPS = [
            ((0, 15, 2), (1, 16, 2), 8),
            ((0, 2, 1), (2, 4, 1), 2),
            ((4, 6, 1), (6, 8, 1), 2),
            ((8, 10, 1), (10, 12, 1), 2),
            ((12, 14, 1), (14, 16, 1), 2),
            ((1, 14, 4), (2, 15, 4), 4),
            ((0, 4, 3), (4, 8, 3), 2),
            ((8, 12, 3), (12, 16, 3), 2),
            ((1, 3, 1), (5, 7, 1), 2),
            ((9, 11, 1), (13, 15, 1), 2),
            ((0, 8, 7), (8, 16, 7), 2),
            ((2, 4, 1), (4, 6, 1), 2),
            ((10, 12, 1), (12, 14, 1), 2),
            ((1, 6, 2), (2, 7, 2), 3),
            ((9, 14, 2), (10, 15, 2), 3),
            ((1, 7, 1), (9, 15, 1), 6),
            ((4, 8, 1), (8, 12, 1), 4),
            ((2, 4, 1), (4, 6, 1), 2),
            ((6, 8, 1), (8, 10, 1), 2),
            ((10, 12, 1), (12, 14, 1), 2),
            ((1, 14, 2), (2, 15, 2), 7),
        ]

        # Poly rows sort on Pool (add/sub/mult only): min=(a+b)/2-|a-b|/2,
        # max=(a+b)/2+|a-b|/2, |.|/2 on ACT. Exact in f32 (ints < 2^15).
        pf = pksf.ap()
        for (a0, a1, ast), (b0, b1, bst), w in GROUPS:
            A = pf[:, :, a0:a1:ast]
            Bp = pf[:, :, b0:b1:bst]
            tt(gps, sumP.ap()[:, :, 0:w], A, Bp, Alu.add)
            tt(gps, difP.ap()[:, :, 0:w], A, Bp, Alu.subtract)
            act.activation(out=abh.ap()[:, :, 0:w], in_=difP.ap()[:, :, 0:w],
                           func=Act.Abs, bias=0.0, scale=0.5)
            act.activation(out=hsm.ap()[:, :, 0:w], in_=sumP.ap()[:, :, 0:w],
                           func=Act.Identity, bias=0.0, scale=0.5)
            tt(gps, A, hsm.ap()[:, :, 0:w], abh.ap()[:, :, 0:w], Alu.subtract)
            tt(gps, Bp, hsm.ap()[:, :, 0:w], abh.ap()[:, :, 0:w], Alu.add)

        # poly unpack (f32 -> i16 round) + area + b-materialization
        ts(vec, suc.ap()[:, 128:192, :], pksf.ap(),
           1.0 / 129.0, Alu.mult, 128.0 / 129.0 - 0.5, Alu.add)
        aAP = suc.ap()[:, 128:192, 0:16:2]
        bAP = suc.ap()[:, 128:192, 1:16:2]
        tt(vec, dd.ap()[:, 0:64, 0:8], bAP, aAP, Alu.subtract)
        ts(vec, dd.ap()[:, 0:64, 0:8], dd.ap()[:, 0:64, 0:8], 0.0, Alu.add,
           0.0, Alu.add, accum=stats.ap()[:, 3:4])
        act.activation(out=bE.ap(), func=Act.Identity, bias=0.0, scale=1.0,
                       in_=bAP.unsqueeze(3).to_broadcast([K, 64, 8, D]))

        # Disk rows sort on DVE (int16 min/max + staging copy)
        vec.scalar_tensor_tensor(out=pks.ap()[:, :, 0:D], in0=s8.ap(),
                                 scalar=129.0, in1=e8.ap(),
                                 op0=Alu.mult, op1=Alu.subtract)
        p = pks.ap()
        mt = mtA.ap()
        for (a0, a1, ast), (b0, b1, bst), w in GROUPS:
            A = p[:, :, a0:a1:ast]
            Bp = p[:, :, b0:b1:bst]
            tt(vec, mt[:, :, 0:w], A, Bp, Alu.min)
            tt(vec, Bp, A, Bp, Alu.max)
            vec.tensor_copy(out=A, in_=mt[:, :, 0:w])

        # ---- P4: unpack (DVE ts computes in fp32, rounds on int16 out) ----
        ts(vec, sshift.ap(), pks.ap(), 1.0 / 129.0, Alu.mult,
           128.0 / 129.0 - 0.5 - 128.0, Alu.add)
        ts(vec, suc.ap()[:, 0:128, :], pks.ap(), 1.0 / 129.0,
           Alu.mult, 128.0 / 129.0 - 0.5, Alu.add)
        # e-128 = p - 129*s (disk rows)
        vec.scalar_tensor_tensor(out=etld.ap(), in0=suc.ap()[:, 0:128, :],
                                 scalar=-129.0, in1=pks.ap(),
                                 op0=Alu.mult, op1=Alu.add)

        # ---- P5: prefix-max ends -> runs -> disk area ----
        tt(vec, ebuf.ap()[:, :, 1:16], etld.ap()[:, :, 1:16], etld.ap()[:, :, 0:15], Alu.max)
        vec.tensor_copy(out=ebuf.ap()[:, :, 0:1], in_=etld.ap()[:, :, 0:1])
        tt(vec, etld.ap()[:, :, 2:16], ebuf.ap()[:, :, 2:16], ebuf.ap()[:, :, 0:14], Alu.max)
        vec.tensor_copy(out=etld.ap()[:, :, 0:2], in_=ebuf.ap()[:, :, 0:2])
        tt(vec, ebuf.ap()[:, :, 4:16], etld.ap()[:, :, 4:16], etld.ap()[:, :, 0:12], Alu.max)
        vec.tensor_copy(out=ebuf.ap()[:, :, 0:4], in_=etld.ap()[:, :, 0:4])
        tt(vec, etld.ap()[:, :, 8:16], ebuf.ap()[:, :, 8:16], ebuf.ap()[:, :, 0:8], Alu.max)
        vec.tensor_copy(out=etld.ap()[:, :, 0:8], in_=ebuf.ap()[:, :, 0:8])
        # u' = min(Rp_j, s'_{j+1});  darea += relu(u' - s'_j)
        tt(vec, uu.ap(), etld.ap()[:, :, 0:D], sshift.ap()[:, :, 1:16], Alu.min)
        tt(vec, dd.ap(), uu.ap(), sshift.ap()[:, :, 0:D], Alu.subtract)
        ts(vec, dd.ap(), dd.ap(), 0.0, Alu.max, 0.0, Alu.add,
           accum=stats.ap()[:, 0:1])

        # ---- P6: intersection over rows 32:96 (int16, DVE) ----
        sp = sshift.ap()[:, 32:96, 0:D].unsqueeze(2).to_broadcast([K, 64, 8, D])
        up = uu.ap()[:, 32:96, :].unsqueeze(2).to_broadcast([K, 64, 8, D])
        ap_ = aAP.unsqueeze(3).to_broadcast([K, 64, 8, D])
        tt(vec, mx.ap(), sp, ap_, Alu.max)
        tt(vec, mn.ap(), up, bE.ap(), Alu.min)
        tt(vec, df.ap(), mn.ap(), mx.ap(), Alu.subtract)
        ts(vec, df.ap(), df.ap(), 0.0, Alu.max, 0.0, Alu.add,
           accum=stats.ap()[:, 1:2])

        # ---- P7: epilogue ----
        itr = stats.ap()[:, 4:5]; uni = stats.ap()[:, 5:6]
        den = stats.ap()[:, 6:7]; pob = stats.ap()[:, 7:8]
        ts(vec, itr, stats.ap()[:, 1:2], 0.0, Alu.add)
        tt(vec, uni, stats.ap()[:, 0:1], stats.ap()[:, 3:4], Alu.add)
        tt(vec, uni, uni, itr, Alu.subtract)
        ts(vec, den, uni, 1e-6, Alu.add)
        vec.reciprocal(out=den, in_=den)
        tt(vec, pob, itr, den, Alu.mult)
        ts(vec, pob, pob, -1.0, Alu.mult, 1.0, Alu.add)      # 1 - inter/union
        tt(vec, colq.ap()[:, 0:1], pob, maskf.ap(), Alu.mult)
        vec.tensor_copy(out=colq.ap()[:, 1:2], in_=maskf.ap())
        nc.tensor.matmul(out=psum.ap(), lhsT=onesv.ap(), rhs=colq.ap(),
                         start=True, stop=True)
        vec.tensor_copy(out=outsb.ap(), in_=psum.ap())
        nc.sync.dma_start(out_d.ap().unsqueeze(0), outsb.ap())

    nc.compile()
    return nc


def _get_nc():
    if "nc" not in _CACHE:
        _CACHE["nc"] = _build_nc()
    return _CACHE["nc"]


def kernel(output, mask, ind, target, freq_mask=None):
    nc = _get_nc()
    from concourse.bass_utils import run_bass_kernel_spmd

    output = np.asarray(output, dtype=np.float32)
    target = np.asarray(target, dtype=np.float32)
    in_maps = []
    for b in range(B):
        in_maps.append({
            "featT": np.ascontiguousarray(output[b].reshape(C, H * W).T),
            "ind": np.asarray(ind[b], dtype=np.int32),
            "target": np.ascontiguousarray(target[b]),
            "mask": np.asarray(mask[b], dtype=np.int32),
        })
    res = run_bass_kernel_spmd(nc, in_maps, core_ids=list(range(B)))
    parts = np.stack([np.asarray(r["out"], dtype=np.float64) for r in res.results])
    loss = parts[:, 0].sum() / (parts[:, 1].sum() + 1e-6)
    return np.float32(loss), np.float32(0.0)


# revision 28
# speedup vs baseline: 1.2348x; 1.2348x over previous
"""DiskLoss Trainium2 kernel (interval-union formulation).

Computes the reference loss:
  pred = gather(output, ind)            # [K,33] per batch
  gt_m = even-odd rasterization of the 16-gon from target   (per object)
  dk_m = union of 15 disks (radius ceil(|pred[:,32]|)) from pred
  per_obj = 1 - inter/(union+1e-6);  loss = sum(m*per_obj)/(sum(m)+1e-6)

Sharding: data-parallel over batch B=8 -> one batch element per NeuronCore.
Each core reduces its 128 objects (object-per-partition layout) to
(sum m*per_obj, sum m); host adds the 8 partial pairs.

Device algorithm — both masks are per-row interval unions, no pixel raster:
  - disks: per (k,y,d) the row span is [s,e) with h=sqrt(relu(r^2-(y-cy)^2)),
    s=max(0,floor(cx-h)+1), e=min(128,floor(cx+h)+1); floor/clip via ACT
    round-to-nearest int conversion (+-0.5 bias) with uint8 saturation;
    pack p=s*129+(e-128) int16, Batcher-sort the 16 slots per row, unpack
    via ACT, prefix-max ends -> disjoint runs [s'_j, u'_j) (values shifted
    by -128); disk area = sum relu(u'-s')
  - polygon: crossings xint per (y in 32:96, v) in fp32; straddle via
    (y-y1)(y-y2)<0; c-128 = round(xint-95.5) via ACT, zeroed (sentinel)
    unless straddle, packed *130 and sorted in the same Batcher pass
    (rows 128:192); even-odd pairs (a_i,b_i) of the sorted crossings are
    the disjoint poly intervals; poly area = sum(b-a)
  - intersection = sum_{i,j} relu(min(b_i,u'_j) - max(a_i,s'_j)) over the
    poly-overlap rows 32:96 (8 poly pairs x 15 disk runs, int16)
  - DVE does min/max/sort; Pool (gpsimd) does arithmetic (add/sub/mult
    only on this ISA); ACT does all float->int rounding; PE does the final
    masked reduction via ones-matmul over partitions
"""

import sys

if "/opt/trn_rl_repo" not in sys.path:
    sys.path.insert(0, "/opt/trn_rl_repo")

import numpy as np

B, C, H, W = 8, 33, 128, 128
K = 128
V = 16          # polygon vertices
D = 15          # disk centers

_CACHE = {}


def _build_nc():
    import concourse.bacc as bacc
    import concourse.mybir as mybir
    import concourse.tile as tile
    import concourse.bass as bass

    F32 = mybir.dt.float32
    F16 = mybir.dt.float16
    I32 = mybir.dt.int32
    I16 = mybir.dt.int16
    U8 = mybir.dt.uint8
    Alu = mybir.AluOpType
    Act = mybir.ActivationFunctionType

    nc = bacc.Bacc("TRN2", target_bir_lowering=False, debug=False)

    # ---- DRAM I/O (per core) ----
    featT_d = nc.dram_tensor("featT", [H * W, C], F32, kind="ExternalInput")
    ind_d = nc.dram_tensor("ind", [K], I32, kind="ExternalInput")
    tgt_d = nc.dram_tensor("target", [K, C], F32, kind="ExternalInput")
    mask_d = nc.dram_tensor("mask", [K], I32, kind="ExternalInput")
    out_d = nc.dram_tensor("out", [2], F32, kind="ExternalOutput")

    # ---- SBUF ----
    pred = nc.alloc_sbuf_tensor("pred", [K, C], F32)
    tgt = nc.alloc_sbuf_tensor("tgt", [K, C], F32)
    indc = nc.alloc_sbuf_tensor("indc", [K, 1], I32)
    maski = nc.alloc_sbuf_tensor("maski", [K, 1], I32)
    maskf = nc.alloc_sbuf_tensor("maskf", [K, 1], F32)

    pxi = nc.alloc_sbuf_tensor("pxi", [K, W], I32)
    pyg = nc.alloc_sbuf_tensor("pyg", [K, W], F32)      # y global 0..127
    pysh = nc.alloc_sbuf_tensor("pysh", [K, 64], F32)   # y-32 for rows 32:96

    # disk geometry
    negcu = nc.alloc_sbuf_tensor("negcu", [K, D], F32)  # -(cy+32)
    cxg = nc.alloc_sbuf_tensor("cxg", [K, D], F32)      # cx+32
    rsc = nc.alloc_sbuf_tensor("rsc", [K, 4], F32)
    ri = nc.alloc_sbuf_tensor("ri", [K, 1], I32)
    r2u = nc.alloc_sbuf_tensor("r2u", [K, 1], F32)
    sqyu = nc.alloc_sbuf_tensor("sqyu", [K, H, D], F32)
    hsq = nc.alloc_sbuf_tensor("hsq", [K, H, D], F32)
    hh = nc.alloc_sbuf_tensor("hh", [K, H, D], F32)
    lo = nc.alloc_sbuf_tensor("lo", [K, H, D], F32)
    hi = nc.alloc_sbuf_tensor("hi", [K, H, D], F32)
    s8 = nc.alloc_sbuf_tensor("s8", [K, H, D], U8)
    e8 = nc.alloc_sbuf_tensor("e8", [K, H, D], U8)

    # polygon geometry
    x2b = nc.alloc_sbuf_tensor("x2b", [K, V], F32)
    y2b = nc.alloc_sbuf_tensor("y2b", [K, V], F32)
    pv1 = nc.alloc_sbuf_tensor("pv1", [K, V], F32)
    pv2 = nc.alloc_sbuf_tensor("pv2", [K, V], F32)
    pv3 = nc.alloc_sbuf_tensor("pv3", [K, V], F32)
    xa = nc.alloc_sbuf_tensor("xa", [K, 64, V], F32)
    ya = nc.alloc_sbuf_tensor("ya", [K, 64, V], F32)
    xb = nc.alloc_sbuf_tensor("xb", [K, 64, V], F32)
    nei = nc.alloc_sbuf_tensor("nei", [K, 64, V], I16)
    cpre = nc.alloc_sbuf_tensor("cpre", [K, 64, V], I16)

    # sort + runs (all int16, DVE)
    pks = nc.alloc_sbuf_tensor("pks", [K, 192, 16], I16)
    mtA = nc.alloc_sbuf_tensor("mtA", [K, 192, 8], I16)
    sshift = nc.alloc_sbuf_tensor("sshift", [K, H, 16], I16)   # s-128 (disk)
    suc = nc.alloc_sbuf_tensor("suc", [K, 192, 16], I16)       # s | c-128
    etld = nc.alloc_sbuf_tensor("etld", [K, H, 16], I16)       # e-128 sorted
    ebuf = nc.alloc_sbuf_tensor("ebuf", [K, H, 16], I16)
    uu = nc.alloc_sbuf_tensor("uu", [K, H, D], I16)
    dd = nc.alloc_sbuf_tensor("dd", [K, H, D], I16)

    # intersection
    mx = nc.alloc_sbuf_tensor("mx", [K, 64, 8, D], I16)
    mn = nc.alloc_sbuf_tensor("mn", [K, 64, 8, D], I16)
    df = nc.alloc_sbuf_tensor("df", [K, 64, 8, D], I16)
    aE = nc.alloc_sbuf_tensor("aE", [K, 64, 8, D], I16)
    bE = nc.alloc_sbuf_tensor("bE", [K, 64, 8, D], I16)

    # act bias constants
    bm95 = nc.alloc_sbuf_tensor("bm95", [K, 1], F32)     # -95.5 (poly c shift)
    bp05 = nc.alloc_sbuf_tensor("bp05", [K, 1], F32)     # +0.5
    b1275 = nc.alloc_sbuf_tensor("b1275", [K, 1], F32)   # +127.5

    # reduction
    stats = nc.alloc_sbuf_tensor("stats", [K, 8], F32)
    onesv = nc.alloc_sbuf_tensor("onesv", [K, 1], F32)
    colq = nc.alloc_sbuf_tensor("colq", [K, 2], F32)
    outsb = nc.alloc_sbuf_tensor("outsb", [1, 2], F32)
    psum = nc.alloc_psum_tensor("psum", [1, 2], F32)

    with tile.TileContext(nc) as tc:
        vec = nc.vector
        gps = nc.gpsimd
        act = nc.scalar

        def ts(eng, out, in0, s1, op0, s2=None, op1=None, accum=None):
            kw = {}
            if accum is not None:
                kw["accum_out"] = accum
            if op1 is not None:
                return eng.tensor_scalar(out=out, in0=in0, scalar1=s1, scalar2=s2,
                                         op0=op0, op1=op1, **kw)
            return eng.tensor_scalar(out=out, in0=in0, scalar1=s1, scalar2=None,
                                     op0=op0, **kw)

        def tt(eng, out, in0, in1, op):
            return eng.tensor_tensor(out=out, in0=in0, in1=in1, op=op)

        # ---- P0: input DMAs + gather + iotas + consts ----
        nc.sync.dma_start(indc.ap(), ind_d.ap().unsqueeze(1))
        nc.sync.dma_start(tgt.ap(), tgt_d.ap())
        nc.sync.dma_start(maski.ap(), mask_d.ap().unsqueeze(1))
        nc.gpsimd.indirect_dma_start(
            out=pred.ap(), out_offset=None, in_=featT_d.ap(),
            in_offset=bass.IndirectOffsetOnAxis(ap=indc.ap(), axis=0))

        nc.gpsimd.iota(pxi.ap(), pattern=[[1, W]], base=0, channel_multiplier=0)
        ts(vec, pyg.ap(), pxi.ap(), 0.0, Alu.add)            # int->f32, 0..127
        ts(vec, pysh.ap(), pxi.ap()[:, 32:96], -32.0, Alu.add)
        ts(vec, maskf.ap(), maski.ap(), 0.0, Alu.add)
        vec.memset(bm95.ap(), -95.5)
        vec.memset(bp05.ap(), 0.5)
        vec.memset(b1275.ap(), 127.5)
        vec.memset(onesv.ap(), 1.0)

        # ---- P2a: disk scalars first (unblocks the ACT Square chain) ----
        ts(vec, negcu.ap(), pred.ap()[:, 1:2 * D:2], -1.0, Alu.mult, -32.0, Alu.add)
        ts(vec, cxg.ap(), pred.ap()[:, 0:2 * D:2], 32.0, Alu.add)
        u = rsc.ap()[:, 0:1]; t = rsc.ap()[:, 1:2]; g = rsc.ap()[:, 2:3]
        ts(vec, t, pred.ap()[:, 32:33], -1.0, Alu.mult)
        tt(vec, u, pred.ap()[:, 32:33], t, Alu.max)          # |p|
        vec.tensor_copy(out=ri.ap(), in_=u)
        vec.tensor_copy(out=t, in_=ri.ap())
        tt(vec, g, t, u, Alu.is_gt)
        tt(vec, t, t, g, Alu.subtract)                       # floor
        tt(vec, g, u, t, Alu.is_gt)
        tt(vec, t, t, g, Alu.add)                            # ceil = r
        tt(vec, r2u.ap(), t, t, Alu.mult)                    # r^2

        for d in range(D):
            act.activation(out=sqyu.ap()[:, :, d], in_=pyg.ap(), func=Act.Square,
                           bias=negcu.ap()[:, d:d + 1], scale=1.0)

        # ---- P1: polygon precompute (raw coords; rows are y-32 in 0..63) ----
        x1v = tgt.ap()[:, 0:2 * V:2]
        y1v = tgt.ap()[:, 1:2 * V:2]
        gps.tensor_copy(out=x2b.ap()[:, 0:V - 1], in_=tgt.ap()[:, 2:2 * V:2])
        gps.tensor_copy(out=x2b.ap()[:, V - 1:V], in_=tgt.ap()[:, 0:1])
        gps.tensor_copy(out=y2b.ap()[:, 0:V - 1], in_=tgt.ap()[:, 3:2 * V:2])
        gps.tensor_copy(out=y2b.ap()[:, V - 1:V], in_=tgt.ap()[:, 1:2])
        d0 = pv1.ap(); eqz = pv2.ap(); sl = pv3.ap()
        tt(vec, d0, y2b.ap(), y1v, Alu.subtract)
        ts(vec, eqz, d0, 0.0, Alu.is_equal)
        tt(vec, d0, d0, eqz, Alu.add)                        # denom
        vec.reciprocal(out=eqz, in_=d0)                      # 1/denom
        tt(vec, sl, x2b.ap(), x1v, Alu.subtract)
        tt(vec, sl, sl, eqz, Alu.mult)                       # slope

        pyb = pysh.ap().unsqueeze(2).to_broadcast([K, 64, V])
        y1b = y1v.unsqueeze(1).to_broadcast([K, 64, V])
        y2bb = y2b.ap().unsqueeze(1).to_broadcast([K, 64, V])
        # straddle = (y-y1)(y-y2) < 0  (a.s. equal to reference predicate)
        tt(vec, xa.ap(), pyb, y1b, Alu.subtract)             # y-y1 (also xint)
        tt(gps, ya.ap(), pyb, y2bb, Alu.subtract)            # y-y2
        tt(gps, ya.ap(), ya.ap(), xa.ap(), Alu.mult)
        ts(vec, nei.ap(), ya.ap(), 0.0, Alu.is_lt)           # straddle 0/1 i16
        # xint (raw coords) = x1 + (y - y1)*slope
        tt(vec, xb.ap(), xa.ap(), sl.unsqueeze(1).to_broadcast([K, 64, V]), Alu.mult)
        tt(vec, xa.ap(), xb.ap(), x1v.unsqueeze(1).to_broadcast([K, 64, V]), Alu.add)
        # c-128 = round(xint_raw - 95.5); garbage for non-straddle (zeroed next)
        act.activation(out=cpre.ap(), in_=xa.ap(), func=Act.Identity,
                       bias=bm95.ap(), scale=1.0)
        # pack poly rows: p = (c-128)*130 * straddle
        vec.scalar_tensor_tensor(out=pks.ap()[:, 128:192, :], in0=cpre.ap(),
                                 scalar=130.0, in1=nei.ap(),
                                 op0=Alu.mult, op1=Alu.mult)

        # ---- P2b: disk per-row geometry ----
        # hsqn = min(sqyu - r^2, 0);  h = sqrt(-hsqn)
        ts(vec, hsq.ap(), sqyu.ap(), r2u.ap(), Alu.subtract, 0.0, Alu.min)
        act.activation(out=hh.ap(), in_=hsq.ap(), func=Act.Sqrt,
                       bias=0.0, scale=-1.0)
        cxb = cxg.ap().unsqueeze(1).to_broadcast([K, H, D])
        tt(vec, lo.ap(), cxb, hh.ap(), Alu.subtract)
        tt(vec, hi.ap(), cxb, hh.ap(), Alu.add)
        # s = round(lo+0.5) sat to [0,255];  e_rev = round(127.5-hi) sat
        act.activation(out=s8.ap(), in_=lo.ap(), func=Act.Identity,
                       bias=bp05.ap(), scale=1.0)
        act.activation(out=e8.ap(), in_=hi.ap(), func=Act.Identity,
                       bias=b1275.ap(), scale=-1.0)
        gps.memset(pks.ap()[:, 0:128, D:16], 16512)          # s=128,e=128 sentinel

        # ---- P3: Batcher odd-even mergesort of the 16 slots (all rows) ----
        GROUPS = [
            ((0, 15, 2), (1, 16, 2), 8),
            ((0, 2, 1), (2, 4, 1), 2),
            ((4, 6, 1), (6, 8, 1), 2),
            ((8, 10, 1), (10, 12, 1), 2),
            ((12, 14, 1), (14, 16, 1), 2),
            ((1, 14, 4), (2, 15, 4), 4),
            ((0, 4, 3), (4, 8, 3), 2),
            ((8, 12, 3), (12, 16, 3), 2),
            ((1, 3, 1), (5, 7, 1), 2),
            ((9, 11, 1), (13, 15, 1), 2),
            ((0, 8, 7), (8, 16, 7), 2),
            ((2, 4, 1), (4, 6, 1), 2),
            ((10, 12, 1), (12, 14, 1), 2),
            ((1, 6, 2), (2, 7, 2), 3),
            ((9, 14, 2), (10, 15, 2), 3),
            ((1, 7, 1), (9, 15, 1), 6),
            ((4, 8, 1), (8, 12, 1), 4),
            ((2, 4, 1), (4, 6, 1), 2),
            ((6, 8, 1), (8, 10, 1), 2),
            ((10, 12, 1), (12, 14, 1), 2),
            ((1, 14, 2), (2, 15, 2), 7),
        ]

        def emit_sort(r0, r1):
            p = pks.ap()[:, r0:r1, :]
            mt = mtA.ap()[:, r0:r1, :]
            for (a0, a1, ast), (b0, b1, bst), w in GROUPS:
                A = p[:, :, a0:a1:ast]
                Bp = p[:, :, b0:b1:bst]
                tt(vec, mt[:, :, 0:w], A, Bp, Alu.min)
                tt(vec, Bp, A, Bp, Alu.max)
                vec.tensor_copy(out=A, in_=mt[:, :, 0:w])

        # poly rows sort first (ready ~15us before the disk rows), then the
        # poly-side unpack + operand materialization overlap the disk sort
        emit_sort(128, 192)
        ts(vec, suc.ap()[:, 128:192, :], pks.ap()[:, 128:192, :],
           1.0 / 129.0, Alu.mult, 128.0 / 129.0 - 0.5, Alu.add)
        aAP = suc.ap()[:, 128:192, 0:16:2]
        bAP = suc.ap()[:, 128:192, 1:16:2]
        tt(vec, dd.ap()[:, 0:64, 0:8], bAP, aAP, Alu.subtract)
        ts(vec, dd.ap()[:, 0:64, 0:8], dd.ap()[:, 0:64, 0:8], 0.0, Alu.add,
           0.0, Alu.add, accum=stats.ap()[:, 3:4])
        act.activation(out=aE.ap(), func=Act.Identity, bias=0.0, scale=1.0,
                       in_=aAP.unsqueeze(3).to_broadcast([K, 64, 8, D]))
        act.activation(out=bE.ap(), func=Act.Identity, bias=0.0, scale=1.0,
                       in_=bAP.unsqueeze(3).to_broadcast([K, 64, 8, D]))

        # pack disk rows: p = s*129 - e_rev = s*129 + (e-128)
        vec.scalar_tensor_tensor(out=pks.ap()[:, 0:128, 0:D], in0=s8.ap(),
                                 scalar=129.0, in1=e8.ap(),
                                 op0=Alu.mult, op1=Alu.subtract)
        emit_sort(0, 128)

        # ---- P4: unpack (DVE ts computes in fp32, rounds on int16 out) ----
        ts(vec, sshift.ap(), pks.ap()[:, 0:128, :], 1.0 / 129.0, Alu.mult,
           128.0 / 129.0 - 0.5 - 128.0, Alu.add)
        ts(vec, suc.ap()[:, 0:128, :], pks.ap()[:, 0:128, :], 1.0 / 129.0,
           Alu.mult, 128.0 / 129.0 - 0.5, Alu.add)
        # e-128 = p - 129*s (disk rows)
        vec.scalar_tensor_tensor(out=etld.ap(), in0=suc.ap()[:, 0:128, :],
                                 scalar=-129.0, in1=pks.ap()[:, 0:128, :],
                                 op0=Alu.mult, op1=Alu.add)

        # ---- P5: prefix-max ends -> runs -> disk area ----
        tt(vec, ebuf.ap()[:, :, 1:16], etld.ap()[:, :, 1:16], etld.ap()[:, :, 0:15], Alu.max)
        vec.tensor_copy(out=ebuf.ap()[:, :, 0:1], in_=etld.ap()[:, :, 0:1])
        tt(vec, etld.ap()[:, :, 2:16], ebuf.ap()[:, :, 2:16], ebuf.ap()[:, :, 0:14], Alu.max)
        vec.tensor_copy(out=etld.ap()[:, :, 0:2], in_=ebuf.ap()[:, :, 0:2])
        tt(vec, ebuf.ap()[:, :, 4:16], etld.ap()[:, :, 4:16], etld.ap()[:, :, 0:12], Alu.max)
        vec.tensor_copy(out=ebuf.ap()[:, :, 0:4], in_=etld.ap()[:, :, 0:4])
        tt(vec, etld.ap()[:, :, 8:16], ebuf.ap()[:, :, 8:16], ebuf.ap()[:, :, 0:8], Alu.max)
        vec.tensor_copy(out=etld.ap()[:, :, 0:8], in_=ebuf.ap()[:, :, 0:8])
        # u' = min(Rp_j, s'_{j+1});  darea += relu(u' - s'_j)
        tt(vec, uu.ap(), etld.ap()[:, :, 0:D], sshift.ap()[:, :, 1:16], Alu.min)
        tt(vec, dd.ap(), uu.ap(), sshift.ap()[:, :, 0:D], Alu.subtract)
        ts(vec, dd.ap(), dd.ap(), 0.0, Alu.max, 0.0, Alu.add,
           accum=stats.ap()[:, 0:1])

        # ---- P6: intersection over rows 32:96 (int16, DVE) ----
        sp = sshift.ap()[:, 32:96, 0:D].unsqueeze(2).to_broadcast([K, 64, 8, D])
        up = uu.ap()[:, 32:96, :].unsqueeze(2).to_broadcast([K, 64, 8, D])
        tt(vec, mx.ap(), sp, aE.ap(), Alu.max)
        tt(vec, mn.ap(), up, bE.ap(), Alu.min)
        tt(vec, df.ap(), mn.ap(), mx.ap(), Alu.subtract)
        ts(vec, df.ap(), df.ap(), 0.0, Alu.max, 0.0, Alu.add,
           accum=stats.ap()[:, 1:2])

        # ---- P7: epilogue ----
        itr = stats.ap()[:, 4:5]; uni = stats.ap()[:, 5:6]
        den = stats.ap()[:, 6:7]; pob = stats.ap()[:, 7:8]
        ts(vec, itr, stats.ap()[:, 1:2], 0.0, Alu.add)
        tt(vec, uni, stats.ap()[:, 0:1], stats.ap()[:, 3:4], Alu.add)
        tt(vec, uni, uni, itr, Alu.subtract)
        ts(vec, den, uni, 1e-6, Alu.add)
        vec.reciprocal(out=den, in_=den)
        tt(vec, pob, itr, den, Alu.mult)
        ts(vec, pob, pob, -1.0, Alu.mult, 1.0, Alu.add)      # 1 - inter/union
        tt(vec, colq.ap()[:, 0:1], pob, maskf.ap(), Alu.mult)
        vec.tensor_copy(out=colq.ap()[:, 1:2], in_=maskf.ap())
        nc.tensor.matmul(out=psum.ap(), lhsT=onesv.ap(), rhs=colq.ap(),
                         start=True, stop=True)
        vec.tensor_copy(out=outsb.ap(), in_=psum.ap())
        nc.sync.dma_start(out_d.ap().unsqueeze(0), outsb.ap())

    nc.compile()
    return nc


def _get_nc():
    if "nc" not in _CACHE:
        _CACHE["nc"] = _build_nc()
    return _CACHE["nc"]


def kernel(output, mask, ind, target, freq_mask=None):
    nc = _get_nc()
    from concourse.bass_utils import run_bass_kernel_spmd

    output = np.asarray(output, dtype=np.float32)
    target = np.asarray(target, dtype=np.float32)
    in_maps = []
    for b in range(B):
        in_maps.append({
            "featT": np.ascontiguousarray(output[b].reshape(C, H * W).T),
            "ind": np.asarray(ind[b], dtype=np.int32),
            "target": np.ascontiguousarray(target[b]),
            "mask": np.asarray(mask[b], dtype=np.int32),
        })
    res = run_bass_kernel_spmd(nc, in_maps, core_ids=list(range(B)))
    parts = np.stack([np.asarray(r["out"], dtype=np.float64) for r in res.results])
    loss = parts[:, 0].sum() / (parts[:, 1].sum() + 1e-6)
    return np.float32(loss), np.float32(0.0)
